# revision 17
# baseline (speedup 1.0000x reference)
"""Trainium2 Bass kernel for nn_HNN_layer (dense_mlp, memory regime).

Math: the reference never increments start_i, so every block reads
x[:, 0:fn] with fn <= 13.  The whole module collapses to

    out = sigmoid(relu(x[:, :13] @ W + b) @ fk + fb)          (B, 1)

with W a (13, 22) matrix packed from `kernels` (column i holds
kernels[off_i : off_i+fn_i], zero-padded), b = biases, fk = final_kernel,
fb = final_bias.

Device strategy (pure data parallel over 8 cores, Bc = 131072 rows/core,
padded to 52 bursts x 5 groups x 512 columns):
  - MM1: block-diagonal lhsT (65, 110) = 5 copies of W -> h for 5 batch
    groups per 512-column burst, PSUM (110, 512).
  - h-exit: relu(h + b) fused into the PSUM->SBUF move (bf16 out, FD=1024),
    alternating ScalarE activation / VectorE tensor_scalar to balance
    the two engines.
  - MM2: block-diagonal lhsT (110, 5) = fk per group; the output column
    strip cycles over PSUM partition bases {0,32,64,96} so four bursts of
    z land in one (128, 512) PSUM bank and a single sigmoid activation
    reads a 101-partition-dense tile (vs 5-partition-sparse).
  - sigmoid(z + fb) fused into the z PSUM->SBUF exit (fp32 out), one
    strided DMA per 4-burst macro back to DRAM.
"""

import sys

if "/opt/trn_rl_repo" not in sys.path:
    sys.path.insert(0, "/opt/trn_rl_repo")

from contextlib import ExitStack

import numpy as np
import ml_dtypes

import concourse.bass as bass
import concourse.bacc as bacc
import concourse.mybir as mybir
import concourse.tile as tile
from concourse.bass_utils import run_bass_kernel_spmd

FEATURE_LIST = [10, 13, 13, 7, 3, 6, 3, 13, 5, 4, 6, 4, 5, 4, 4, 5, 4, 3, 3, 7, 3, 3]
NB = len(FEATURE_LIST)  # 22 blocks
FMAX = 13               # max(FEATURE_LIST): only x[:, :13] is ever read
B_TOTAL = 1048576
N_CORES = 8
BC = B_TOTAL // N_CORES       # 131072 rows per core
G = 5                         # batch groups packed per matmul column
N = 512                       # free-dim columns per burst (one PSUM bank)
NBURST = 52                   # ceil(BC / (G*N)) -> padded
Q = NBURST * N                # 26624 padded rows per group
KP = FMAX * G                 # 65  (MM1 contraction)
MP = NB * G                   # 110 (MM1 output partitions / MM2 contraction)
XCHUNKS = [2, 2] + [4] * 12   # bursts per input DMA; tiny first chunks so
                              # compute starts early (waits auto-split by bacc)
ZMACRO = 4                    # bursts of z per PSUM z-bank / sigmoid
OBATCH = 4                    # sigmoid macros per output DMA

BF16 = mybir.dt.bfloat16
F32 = mybir.dt.float32
_BUILD_CACHE = {}


def _dve_exit_flags(n_exits, dve_frac=0.586):
    """Evenly interleaved True(=VectorE)/False(=ScalarE) schedule."""
    n_dve = round(n_exits * dve_frac)
    flags = []
    acc = 0.0
    for _ in range(n_exits):
        acc += n_dve / n_exits
        if acc >= 1.0 - 1e-9:
            flags.append(True)
            acc -= 1.0
        else:
            flags.append(False)
    return flags


def build_program():
    """Build the SPMD Bass program (one NeuronCore's view)."""
    nc = bacc.Bacc("TRN2", target_bir_lowering=False, debug=False)

    xg = nc.dram_tensor("xg", [KP, Q], BF16, kind="ExternalInput").ap()
    # one packed constant tensor -> one DMA (8 HWDGE lanes total, no reuse)
    # layout: [0:65, 0:110]=W1 f32, [0:110, 110:142]=W2 f32,
    #         [0:110, 142]=bias, [0:128, 143]=final_bias
    cst = nc.dram_tensor("cst", [128, 144], F32, kind="ExternalInput").ap()
    # out[c, g, m, n] = sigmoid for group g, burst 4m + c, col n
    outd = nc.dram_tensor("out", [ZMACRO, G, NBURST // ZMACRO, N], F32,
                          kind="ExternalOutput").ap()

    dve_flags = _dve_exit_flags(NBURST // 2)

    with tile.TileContext(nc) as tc, ExitStack() as ctx:
        const = ctx.enter_context(tc.tile_pool(name="const", bufs=1))
        hps_pool = ctx.enter_context(tc.tile_pool(name="hps", bufs=3, space="PSUM"))
        zps_pool = ctx.enter_context(tc.tile_pool(name="zps", bufs=2, space="PSUM"))
        hsb_pool = ctx.enter_context(tc.tile_pool(name="hsb", bufs=26))
        osb_pool = ctx.enter_context(tc.tile_pool(name="osb", bufs=4))

        cst_t = const.tile([128, 144], F32)
        nc.sync.dma_start(cst_t[:], cst[:])
        bv_ap = cst_t[0:MP, 142:143]
        fbv_ap = cst_t[:, 143:144]

        # bf16 weight casts (also absorb the const-DMA wait on DVE)
        w1_t = const.tile([KP, MP], BF16)
        nc.vector.tensor_copy(w1_t[:], cst_t[0:KP, 0:MP])
        w2_t = const.tile([MP, 32], BF16)
        nc.vector.tensor_copy(w2_t[:], cst_t[0:MP, 110:142])

        # ACT warmup: absorbs the const-DMA wait on ScalarE and triggers
        # the activation table load before the steady-state loop.
        warm = const.tile([128, 1], F32)
        nc.scalar.activation(
            warm[:], fbv_ap, mybir.ActivationFunctionType.Sigmoid
        )
        nc.scalar.activation(
            warm[:], fbv_ap, mybir.ActivationFunctionType.Relu
        )

        xpools = [
            ctx.enter_context(tc.tile_pool(name=f"x{i}", bufs=1))
            for i in range(len(XCHUNKS))
        ]
        x_starts = [sum(XCHUNKS[:i]) for i in range(len(XCHUNKS))]
        xstate = {"t": None, "s": 0}
        NMAC = NBURST // ZMACRO  # 13 macros of 4 bursts
        hsb_of = {}              # macro -> list of 4 (110, 512) slices
        zps_of = {}              # macro -> zps tile
        osb_state = {"t": None}

        # Software-pipelined macro loop.  PE instruction order is
        #   MM1(m)x4 [w1 stays loaded], MM2(m-1)x4 [w2 stays loaded], ...
        # so the exits of macro m run on ACT/DVE while PE does MM1(m+1),
        # and weights swap twice per macro instead of twice per burst.

        def emit_mm1_group(m):
            slices = []
            for j in range(2):
                hps = hps_pool.tile([MP, 2 * N], F32)
                for k in range(2):
                    t = ZMACRO * m + 2 * j + k
                    if t in x_starts:
                        ci = x_starts.index(t)
                        nb = XCHUNKS[ci]
                        xt = xpools[ci].tile([KP, nb * N], BF16)
                        nc.sync.dma_start(xt[:], xg[:, t * N:(t + nb) * N])
                        xstate["t"], xstate["s"] = xt, t
                    xs = xstate["t"][:, (t - xstate["s"]) * N:
                                     (t - xstate["s"] + 1) * N]
                    nc.tensor.matmul(
                        hps[:, k * N:(k + 1) * N], w1_t[:], xs,
                        start=True, stop=True,
                    )
                slices.append(hps)
            return slices

        def emit_exits(m, hps_tiles):
            out_slices = []
            for j in range(2):
                hsb = hsb_pool.tile([MP, 2 * N], BF16)
                if dve_flags[2 * m + j]:
                    nc.vector.tensor_scalar(
                        hsb[:], hps_tiles[j][:],
                        scalar1=bv_ap, scalar2=0.0,
                        op0=mybir.AluOpType.add, op1=mybir.AluOpType.max,
                    )
                else:
                    nc.scalar.activation(
                        hsb[:], hps_tiles[j][:],
                        mybir.ActivationFunctionType.Relu,
                        bias=bv_ap,
                    )
                out_slices += [hsb[:, 0:N], hsb[:, N:2 * N]]
            hsb_of[m] = out_slices

        def emit_mm2_group(m):
            zps = zps_pool.tile([128, N], F32)
            zps_of[m] = zps
            for c in range(ZMACRO):
                nc.tensor.matmul(
                    zps[32 * c:32 * c + 32, :], w2_t[:], hsb_of[m][c],
                    start=True, stop=True, tile_position=(0, 32 * c),
                )
            del hsb_of[m]

        def emit_sigmoid_out(m):
            zps = zps_of.pop(m)
            if m % OBATCH == 0:
                osb_state["t"] = osb_pool.tile([128, OBATCH * N], F32, name="osb", tag="osb")
            osb = osb_state["t"]
            mo = m % OBATCH
            nc.scalar.activation(
                osb[:, mo * N:(mo + 1) * N], zps[:],
                mybir.ActivationFunctionType.Sigmoid,
                bias=fbv_ap,
            )
            if m % OBATCH == OBATCH - 1 or m == NMAC - 1:
                m0 = (m // OBATCH) * OBATCH
                nm = m - m0 + 1
                for c in range(ZMACRO):
                    src_ap = osb[32 * c:32 * c + G, 0:nm * N].rearrange(
                        "g (mm n) -> g mm n", n=N
                    )
                    nc.scalar.dma_start(outd[c, :, m0:m0 + nm, :], src_ap)

        prev_hps = emit_mm1_group(0)
        emit_exits(0, prev_hps)
        for m in range(1, NMAC):
            hps_tiles = emit_mm1_group(m)
            emit_mm2_group(m - 1)
            emit_sigmoid_out(m - 1)
            emit_exits(m, hps_tiles)
        emit_mm2_group(NMAC - 1)
        emit_sigmoid_out(NMAC - 1)

    nc.compile()
    return nc


def _pack_host_inputs(x, kernels, biases, final_kernel):
    """Build per-core device arrays from the full inputs."""
    W = np.zeros((FMAX, NB), np.float32)
    off = 0
    for i, fn in enumerate(FEATURE_LIST):
        W[:fn, i] = np.asarray(kernels[off:off + fn, 0], np.float32)
        off += fn
    b = np.asarray(biases, np.float32)
    fk = np.asarray(final_kernel[:, 0], np.float32)

    cst = np.zeros((128, 144), np.float32)
    for g in range(G):
        cst[FMAX * g:FMAX * (g + 1), NB * g:NB * (g + 1)] = W       # W1
        cst[NB * g:NB * (g + 1), 110 + g] = fk                      # W2
        cst[NB * g:NB * (g + 1), 142] = b                           # bias

    x13 = np.ascontiguousarray(np.asarray(x[:, :FMAX], np.float32)).astype(
        ml_dtypes.bfloat16
    )
    in_maps = []
    for cidx in range(N_CORES):
        X = np.zeros((KP, Q), ml_dtypes.bfloat16)
        base = cidx * BC
        for g in range(G):
            v = min(Q, BC - g * Q)
            X[FMAX * g:FMAX * g + FMAX, :v] = x13[base + g * Q:base + g * Q + v, :].T
        in_maps.append({"xg": X, "cst": cst})
    return in_maps


def run(x, kernels, biases, final_kernel, final_bias, trace=False, **spmd_kwargs):
    if "nc" not in _BUILD_CACHE:
        _BUILD_CACHE["nc"] = build_program()
    nc = _BUILD_CACHE["nc"]

    fb = float(np.asarray(final_bias).reshape(-1)[0])
    in_maps = _pack_host_inputs(x, kernels, biases, final_kernel)
    for m in in_maps:
        m["cst"][:, 143] = fb
    res = run_bass_kernel_spmd(
        nc, in_maps, list(range(N_CORES)), trace=trace, **spmd_kwargs
    )
    outs = []
    for cidx in range(N_CORES):
        op = np.asarray(res.results[cidx]["out"], np.float32)  # (c, g, m, n)
        arr = op.transpose(1, 2, 0, 3)                         # (g, m, c, n)
        outs.append(arr.reshape(-1)[:BC])
    y = np.concatenate(outs).reshape(B_TOTAL, 1)
    return y, res


def kernel(x, kernels, biases, final_kernel, final_bias):
    y, _ = run(x, kernels, biases, final_kernel, final_bias, trace=False)
    return y


# revision 18
# speedup vs baseline: 1.1543x; 1.1543x over previous
"""Trainium2 Bass kernel for nn_HNN_layer (dense_mlp, memory regime).

Math: the reference never increments start_i, so every block reads
x[:, 0:fn] with fn <= 13.  The whole module collapses to

    out = sigmoid(relu(x[:, :13] @ W + b) @ fk + fb)          (B, 1)

with W a (13, 22) matrix packed from `kernels` (column i holds
kernels[off_i : off_i+fn_i], zero-padded), b = biases, fk = final_kernel,
fb = final_bias.

Device strategy (pure data parallel over 8 cores, Bc = 131072 rows/core,
padded to 52 bursts x 5 groups x 512 columns):
  - MM1: block-diagonal lhsT (65, 110) = 5 copies of W -> h for 5 batch
    groups per 512-column burst, PSUM (110, 512).
  - h-exit: relu(h + b) fused into the PSUM->SBUF move (bf16 out, FD=1024),
    alternating ScalarE activation / VectorE tensor_scalar to balance
    the two engines.
  - MM2: block-diagonal lhsT (110, 5) = fk per group; the output column
    strip cycles over PSUM partition bases {0,32,64,96} so four bursts of
    z land in one (128, 512) PSUM bank and a single sigmoid activation
    reads a 101-partition-dense tile (vs 5-partition-sparse).
  - sigmoid(z + fb) fused into the z PSUM->SBUF exit (fp32 out), one
    strided DMA per 4-burst macro back to DRAM.
"""

import sys

if "/opt/trn_rl_repo" not in sys.path:
    sys.path.insert(0, "/opt/trn_rl_repo")

from contextlib import ExitStack

import numpy as np
import ml_dtypes

import concourse.bass as bass
import concourse.bacc as bacc
import concourse.mybir as mybir
import concourse.tile as tile
from concourse.bass_utils import run_bass_kernel_spmd

FEATURE_LIST = [10, 13, 13, 7, 3, 6, 3, 13, 5, 4, 6, 4, 5, 4, 4, 5, 4, 3, 3, 7, 3, 3]
NB = len(FEATURE_LIST)  # 22 blocks
FMAX = 13               # max(FEATURE_LIST): only x[:, :13] is ever read
B_TOTAL = 1048576
N_CORES = 8
BC = B_TOTAL // N_CORES       # 131072 rows per core
G = 5                         # batch groups packed per matmul column
N = 512                       # free-dim columns per burst (one PSUM bank)
NBURST = 52                   # ceil(BC / (G*N)) -> padded
Q = NBURST * N                # 26624 padded rows per group
KP = FMAX * G                 # 65  (MM1 contraction)
MP = NB * G                   # 110 (MM1 output partitions / MM2 contraction)
XCHUNKS = [2, 2] + [8] * 6    # bursts per input DMA; tiny first chunks so
                              # compute starts early, then big chunks (the
                              # HWDGE trigger costs ~1us on the issuing engine)
ZMACRO = 4                    # bursts of z per PSUM z-bank / sigmoid
OBATCH = 4                    # sigmoid macros per output DMA

BF16 = mybir.dt.bfloat16
F32 = mybir.dt.float32
_BUILD_CACHE = {}


def _dve_exit_flags(n_exits, dve_frac=0.62):
    """Evenly interleaved True(=VectorE)/False(=ScalarE) schedule."""
    n_dve = round(n_exits * dve_frac)
    flags = []
    acc = 0.0
    for _ in range(n_exits):
        acc += n_dve / n_exits
        if acc >= 1.0 - 1e-9:
            flags.append(True)
            acc -= 1.0
        else:
            flags.append(False)
    return flags


def build_program():
    """Build the SPMD Bass program (one NeuronCore's view)."""
    nc = bacc.Bacc("TRN2", target_bir_lowering=False, debug=False)

    xg = nc.dram_tensor("xg", [KP, Q], BF16, kind="ExternalInput").ap()
    # one packed constant tensor -> one DMA (8 HWDGE lanes total, no reuse)
    # layout: [0:65, 0:110]=W1 f32, [0:110, 110:142]=W2 f32,
    #         [0:110, 142]=bias, [0:128, 143]=final_bias
    cst = nc.dram_tensor("cst", [128, 144], F32, kind="ExternalInput").ap()
    # padded layout: out[32c + g, m*N + n] = sigmoid(group g, burst 4m+c)
    # (garbage rows keep each output DMA a single full-partition transfer)
    outd = nc.dram_tensor("out", [128, (NBURST // ZMACRO) * N], F32,
                          kind="ExternalOutput").ap()

    dve_flags = _dve_exit_flags(NBURST // 2)

    with tile.TileContext(nc) as tc, ExitStack() as ctx:
        const = ctx.enter_context(tc.tile_pool(name="const", bufs=1))
        hps_pool = ctx.enter_context(tc.tile_pool(name="hps", bufs=3, space="PSUM"))
        zps_pool = ctx.enter_context(tc.tile_pool(name="zps", bufs=2, space="PSUM"))
        hsb_pool = ctx.enter_context(tc.tile_pool(name="hsb", bufs=26))
        osb_pool = ctx.enter_context(tc.tile_pool(name="osb", bufs=4))

        cst_t = const.tile([128, 144], F32)
        nc.sync.dma_start(cst_t[:], cst[:])
        bv_ap = cst_t[0:MP, 142:143]
        fbv_ap = cst_t[:, 143:144]

        # bf16 weight casts (also absorb the const-DMA wait on DVE)
        w1_t = const.tile([KP, MP], BF16)
        nc.vector.tensor_copy(w1_t[:], cst_t[0:KP, 0:MP])
        w2_t = const.tile([MP, 32], BF16)
        nc.vector.tensor_copy(w2_t[:], cst_t[0:MP, 110:142])

        # ACT warmup: absorbs the const-DMA wait on ScalarE and triggers
        # the activation table load before the steady-state loop.
        warm = const.tile([128, 1], F32)
        nc.scalar.activation(
            warm[:], fbv_ap, mybir.ActivationFunctionType.Sigmoid
        )
        nc.scalar.activation(
            warm[:], fbv_ap, mybir.ActivationFunctionType.Relu
        )

        xpools = [
            ctx.enter_context(tc.tile_pool(name=f"x{i}", bufs=1))
            for i in range(len(XCHUNKS))
        ]
        x_starts = [sum(XCHUNKS[:i]) for i in range(len(XCHUNKS))]
        xstate = {"t": None, "s": 0}
        NMAC = NBURST // ZMACRO  # 13 macros of 4 bursts
        hsb_of = {}              # macro -> list of 4 (110, 512) slices
        zps_of = {}              # macro -> zps tile
        osb_state = {"t": None}

        # Software-pipelined macro loop.  PE instruction order is
        #   MM1(m)x4 [w1 stays loaded], MM2(m-1)x4 [w2 stays loaded], ...
        # so the exits of macro m run on ACT/DVE while PE does MM1(m+1),
        # and weights swap twice per macro instead of twice per burst.

        def emit_mm1_group(m):
            slices = []
            for j in range(2):
                hps = hps_pool.tile([MP, 2 * N], F32)
                for k in range(2):
                    t = ZMACRO * m + 2 * j + k
                    if t in x_starts:
                        ci = x_starts.index(t)
                        nb = XCHUNKS[ci]
                        xt = xpools[ci].tile([KP, nb * N], BF16)
                        nc.sync.dma_start(xt[:], xg[:, t * N:(t + nb) * N])
                        xstate["t"], xstate["s"] = xt, t
                    xs = xstate["t"][:, (t - xstate["s"]) * N:
                                     (t - xstate["s"] + 1) * N]
                    nc.tensor.matmul(
                        hps[:, k * N:(k + 1) * N], w1_t[:], xs,
                        start=True, stop=True,
                    )
                slices.append(hps)
            return slices

        def emit_exits(m, hps_tiles):
            out_slices = []
            for j in range(2):
                hsb = hsb_pool.tile([MP, 2 * N], BF16)
                if dve_flags[2 * m + j]:
                    nc.vector.tensor_scalar(
                        hsb[:], hps_tiles[j][:],
                        scalar1=bv_ap, scalar2=0.0,
                        op0=mybir.AluOpType.add, op1=mybir.AluOpType.max,
                    )
                else:
                    nc.scalar.activation(
                        hsb[:], hps_tiles[j][:],
                        mybir.ActivationFunctionType.Relu,
                        bias=bv_ap,
                    )
                out_slices += [hsb[:, 0:N], hsb[:, N:2 * N]]
            hsb_of[m] = out_slices

        def emit_mm2_group(m):
            zps = zps_pool.tile([128, N], F32)
            zps_of[m] = zps
            for c in range(ZMACRO):
                nc.tensor.matmul(
                    zps[32 * c:32 * c + 32, :], w2_t[:], hsb_of[m][c],
                    start=True, stop=True, tile_position=(0, 32 * c),
                )
            del hsb_of[m]

        def emit_sigmoid_out(m):
            zps = zps_of.pop(m)
            if m % OBATCH == 0:
                osb_state["t"] = osb_pool.tile([128, OBATCH * N], F32, name="osb", tag="osb")
            osb = osb_state["t"]
            mo = m % OBATCH
            nc.scalar.activation(
                osb[:, mo * N:(mo + 1) * N], zps[:],
                mybir.ActivationFunctionType.Sigmoid,
                bias=fbv_ap,
            )
            if m % OBATCH == OBATCH - 1 or m == NMAC - 1:
                m0 = (m // OBATCH) * OBATCH
                nm = m - m0 + 1
                nc.sync.dma_start(
                    outd[:, m0 * N:(m0 + nm) * N], osb[:, 0:nm * N]
                )

        prev_hps = emit_mm1_group(0)
        emit_exits(0, prev_hps)
        for m in range(1, NMAC):
            hps_tiles = emit_mm1_group(m)
            emit_mm2_group(m - 1)
            emit_sigmoid_out(m - 1)
            emit_exits(m, hps_tiles)
        emit_mm2_group(NMAC - 1)
        emit_sigmoid_out(NMAC - 1)

    nc.compile()
    return nc


def _pack_host_inputs(x, kernels, biases, final_kernel):
    """Build per-core device arrays from the full inputs."""
    W = np.zeros((FMAX, NB), np.float32)
    off = 0
    for i, fn in enumerate(FEATURE_LIST):
        W[:fn, i] = np.asarray(kernels[off:off + fn, 0], np.float32)
        off += fn
    b = np.asarray(biases, np.float32)
    fk = np.asarray(final_kernel[:, 0], np.float32)

    cst = np.zeros((128, 144), np.float32)
    for g in range(G):
        cst[FMAX * g:FMAX * (g + 1), NB * g:NB * (g + 1)] = W       # W1
        cst[NB * g:NB * (g + 1), 110 + g] = fk                      # W2
        cst[NB * g:NB * (g + 1), 142] = b                           # bias

    x13 = np.ascontiguousarray(np.asarray(x[:, :FMAX], np.float32)).astype(
        ml_dtypes.bfloat16
    )
    in_maps = []
    for cidx in range(N_CORES):
        X = np.zeros((KP, Q), ml_dtypes.bfloat16)
        base = cidx * BC
        for g in range(G):
            v = min(Q, BC - g * Q)
            X[FMAX * g:FMAX * g + FMAX, :v] = x13[base + g * Q:base + g * Q + v, :].T
        in_maps.append({"xg": X, "cst": cst})
    return in_maps


def run(x, kernels, biases, final_kernel, final_bias, trace=False, **spmd_kwargs):
    if "nc" not in _BUILD_CACHE:
        _BUILD_CACHE["nc"] = build_program()
    nc = _BUILD_CACHE["nc"]

    fb = float(np.asarray(final_bias).reshape(-1)[0])
    in_maps = _pack_host_inputs(x, kernels, biases, final_kernel)
    for m in in_maps:
        m["cst"][:, 143] = fb
    res = run_bass_kernel_spmd(
        nc, in_maps, list(range(N_CORES)), trace=trace, **spmd_kwargs
    )
    outs = []
    nmac = NBURST // ZMACRO
    for cidx in range(N_CORES):
        op = np.asarray(res.results[cidx]["out"], np.float32)  # (128, nmac*N)
        arr = op.reshape(4, 32, nmac, N).transpose(1, 2, 0, 3)  # (r, m, c, n)
        outs.append(arr[:G].reshape(-1)[:BC])
    y = np.concatenate(outs).reshape(B_TOTAL, 1)
    return y, res


def kernel(x, kernels, biases, final_kernel, final_bias):
    y, _ = run(x, kernels, biases, final_kernel, final_bias, trace=False)
    return y


# revision 19
# speedup vs baseline: 1.1646x; 1.0089x over previous
"""Trainium2 Bass kernel for nn_HNN_layer (dense_mlp, memory regime).

Math: the reference never increments start_i, so every block reads
x[:, 0:fn] with fn <= 13.  The whole module collapses to

    out = sigmoid(relu(x[:, :13] @ W + b) @ fk + fb)          (B, 1)

with W a (13, 22) matrix packed from `kernels` (column i holds
kernels[off_i : off_i+fn_i], zero-padded), b = biases, fk = final_kernel,
fb = final_bias.

Device strategy (pure data parallel over 8 cores, Bc = 131072 rows/core,
padded to 52 bursts x 5 groups x 512 columns):
  - MM1: block-diagonal lhsT (65, 110) = 5 copies of W -> h for 5 batch
    groups per 512-column burst, PSUM (110, 512).
  - h-exit: relu(h + b) fused into the PSUM->SBUF move (bf16 out, FD=1024),
    alternating ScalarE activation / VectorE tensor_scalar to balance
    the two engines.
  - MM2: block-diagonal lhsT (110, 5) = fk per group; the output column
    strip cycles over PSUM partition bases {0,32,64,96} so four bursts of
    z land in one (128, 512) PSUM bank and a single sigmoid activation
    reads a 101-partition-dense tile (vs 5-partition-sparse).
  - sigmoid(z + fb) fused into the z PSUM->SBUF exit (fp32 out), one
    strided DMA per 4-burst macro back to DRAM.
"""

import sys

if "/opt/trn_rl_repo" not in sys.path:
    sys.path.insert(0, "/opt/trn_rl_repo")

from contextlib import ExitStack

import numpy as np
import ml_dtypes

import concourse.bass as bass
import concourse.bacc as bacc
import concourse.mybir as mybir
import concourse.tile as tile
from concourse.bass_utils import run_bass_kernel_spmd

FEATURE_LIST = [10, 13, 13, 7, 3, 6, 3, 13, 5, 4, 6, 4, 5, 4, 4, 5, 4, 3, 3, 7, 3, 3]
NB = len(FEATURE_LIST)  # 22 blocks
FMAX = 13               # max(FEATURE_LIST): only x[:, :13] is ever read
B_TOTAL = 1048576
N_CORES = 8
BC = B_TOTAL // N_CORES       # 131072 rows per core
G = 5                         # batch groups packed per matmul column
N = 512                       # free-dim columns per burst (one PSUM bank)
NBURST = 52                   # ceil(BC / (G*N)) -> padded
Q = NBURST * N                # 26624 padded rows per group
KP = FMAX * G                 # 65  (MM1 contraction)
MP = NB * G                   # 110 (MM1 output partitions / MM2 contraction)
XCHUNKS = [2, 2] + [8] * 6    # bursts per input DMA; tiny first chunks so
                              # compute starts early, then big chunks (the
                              # HWDGE trigger costs ~1us on the issuing engine)
ZMACRO = 4                    # bursts of z per PSUM z-bank / sigmoid
OBATCH = 4                    # sigmoid macros per output DMA

BF16 = mybir.dt.bfloat16
F32 = mybir.dt.float32
_BUILD_CACHE = {}


def _dve_exit_flags(n_exits, dve_frac=0.62):
    """Evenly interleaved True(=VectorE)/False(=ScalarE) schedule."""
    n_dve = round(n_exits * dve_frac)
    flags = []
    acc = 0.0
    for _ in range(n_exits):
        acc += n_dve / n_exits
        if acc >= 1.0 - 1e-9:
            flags.append(True)
            acc -= 1.0
        else:
            flags.append(False)
    return flags


def build_program():
    """Build the SPMD Bass program (one NeuronCore's view)."""
    nc = bacc.Bacc("TRN2", target_bir_lowering=False, debug=False)

    xg = nc.dram_tensor("xg", [KP, Q], BF16, kind="ExternalInput").ap()
    w1d = nc.dram_tensor("w1d", [KP, MP], BF16, kind="ExternalInput").ap()
    w2d = nc.dram_tensor("w2d", [MP, 32], BF16, kind="ExternalInput").ap()
    # cst: [0:110, 0]=bias, [0:128, 1]=final_bias
    cst = nc.dram_tensor("cst", [128, 2], F32, kind="ExternalInput").ap()
    # padded layout: out[32c + g, m*N + n] = sigmoid(group g, burst 4m+c)
    # (garbage rows keep each output DMA a single full-partition transfer)
    outd = nc.dram_tensor("out", [128, (NBURST // ZMACRO) * N], F32,
                          kind="ExternalOutput").ap()

    dve_flags = _dve_exit_flags(NBURST // 2)

    with tile.TileContext(nc) as tc, ExitStack() as ctx:
        const = ctx.enter_context(tc.tile_pool(name="const", bufs=1))
        hps_pool = ctx.enter_context(tc.tile_pool(name="hps", bufs=3, space="PSUM"))
        zps_pool = ctx.enter_context(tc.tile_pool(name="zps", bufs=2, space="PSUM"))
        hsb_pool = ctx.enter_context(tc.tile_pool(name="hsb", bufs=26))
        osb_pool = ctx.enter_context(tc.tile_pool(name="osb", bufs=4))

        w1_t = const.tile([KP, MP], BF16)
        nc.sync.dma_start(w1_t[:], w1d[:])
        w2_t = const.tile([MP, 32], BF16)
        nc.sync.dma_start(w2_t[:], w2d[:])
        cst_t = const.tile([128, 2], F32)
        nc.sync.dma_start(cst_t[:], cst[:])
        bv_ap = cst_t[0:MP, 0:1]
        fbv_ap = cst_t[:, 1:2]

        # ACT warmup: absorbs the const-DMA wait on ScalarE and triggers
        # the activation table load before the steady-state loop.
        warm = const.tile([128, 1], F32)
        nc.scalar.activation(
            warm[:], fbv_ap, mybir.ActivationFunctionType.Sigmoid
        )
        nc.scalar.activation(
            warm[:], fbv_ap, mybir.ActivationFunctionType.Relu
        )

        xpools = [
            ctx.enter_context(tc.tile_pool(name=f"x{i}", bufs=1))
            for i in range(len(XCHUNKS))
        ]
        x_starts = [sum(XCHUNKS[:i]) for i in range(len(XCHUNKS))]
        xstate = {"t": None, "s": 0}
        NMAC = NBURST // ZMACRO  # 13 macros of 4 bursts
        hsb_of = {}              # macro -> list of 4 (110, 512) slices
        zps_of = {}              # macro -> zps tile
        osb_state = {"t": None}

        # Software-pipelined macro loop.  PE instruction order is
        #   MM1(m)x4 [w1 stays loaded], MM2(m-1)x4 [w2 stays loaded], ...
        # so the exits of macro m run on ACT/DVE while PE does MM1(m+1),
        # and weights swap twice per macro instead of twice per burst.

        def emit_mm1_group(m):
            slices = []
            for j in range(2):
                hps = hps_pool.tile([MP, 2 * N], F32)
                for k in range(2):
                    t = ZMACRO * m + 2 * j + k
                    if t in x_starts:
                        ci = x_starts.index(t)
                        nb = XCHUNKS[ci]
                        xt = xpools[ci].tile([KP, nb * N], BF16)
                        nc.sync.dma_start(xt[:], xg[:, t * N:(t + nb) * N])
                        xstate["t"], xstate["s"] = xt, t
                    xs = xstate["t"][:, (t - xstate["s"]) * N:
                                     (t - xstate["s"] + 1) * N]
                    nc.tensor.matmul(
                        hps[:, k * N:(k + 1) * N], w1_t[:], xs,
                        start=True, stop=True,
                    )
                slices.append(hps)
            return slices

        def emit_exits(m, hps_tiles):
            out_slices = []
            for j in range(2):
                hsb = hsb_pool.tile([MP, 2 * N], BF16)
                if dve_flags[2 * m + j]:
                    nc.vector.tensor_scalar(
                        hsb[:], hps_tiles[j][:],
                        scalar1=bv_ap, scalar2=0.0,
                        op0=mybir.AluOpType.add, op1=mybir.AluOpType.max,
                    )
                else:
                    nc.scalar.activation(
                        hsb[:], hps_tiles[j][:],
                        mybir.ActivationFunctionType.Relu,
                        bias=bv_ap,
                    )
                out_slices += [hsb[:, 0:N], hsb[:, N:2 * N]]
            hsb_of[m] = out_slices

        def emit_mm2_group(m):
            zps = zps_pool.tile([128, N], F32)
            zps_of[m] = zps
            for c in range(ZMACRO):
                nc.tensor.matmul(
                    zps[32 * c:32 * c + 32, :], w2_t[:], hsb_of[m][c],
                    start=True, stop=True, tile_position=(0, 32 * c),
                )
            del hsb_of[m]

        def emit_sigmoid_out(m):
            zps = zps_of.pop(m)
            if m % OBATCH == 0:
                osb_state["t"] = osb_pool.tile([128, OBATCH * N], F32, name="osb", tag="osb")
            osb = osb_state["t"]
            mo = m % OBATCH
            nc.scalar.activation(
                osb[:, mo * N:(mo + 1) * N], zps[:],
                mybir.ActivationFunctionType.Sigmoid,
                bias=fbv_ap,
            )
            if m % OBATCH == OBATCH - 1 or m == NMAC - 1:
                m0 = (m // OBATCH) * OBATCH
                nm = m - m0 + 1
                nc.sync.dma_start(
                    outd[:, m0 * N:(m0 + nm) * N], osb[:, 0:nm * N]
                )

        prev_hps = emit_mm1_group(0)
        emit_exits(0, prev_hps)
        for m in range(1, NMAC):
            hps_tiles = emit_mm1_group(m)
            emit_mm2_group(m - 1)
            emit_sigmoid_out(m - 1)
            emit_exits(m, hps_tiles)
        emit_mm2_group(NMAC - 1)
        emit_sigmoid_out(NMAC - 1)

    nc.compile()
    return nc


def _pack_host_inputs(x, kernels, biases, final_kernel):
    """Build per-core device arrays from the full inputs."""
    W = np.zeros((FMAX, NB), np.float32)
    off = 0
    for i, fn in enumerate(FEATURE_LIST):
        W[:fn, i] = np.asarray(kernels[off:off + fn, 0], np.float32)
        off += fn
    b = np.asarray(biases, np.float32)
    fk = np.asarray(final_kernel[:, 0], np.float32)

    w1 = np.zeros((KP, MP), np.float32)
    w2 = np.zeros((MP, 32), np.float32)
    cst = np.zeros((128, 2), np.float32)
    for g in range(G):
        w1[FMAX * g:FMAX * (g + 1), NB * g:NB * (g + 1)] = W
        w2[NB * g:NB * (g + 1), g] = fk
        cst[NB * g:NB * (g + 1), 0] = b
    w1 = w1.astype(ml_dtypes.bfloat16)
    w2 = w2.astype(ml_dtypes.bfloat16)

    x13 = np.ascontiguousarray(np.asarray(x[:, :FMAX], np.float32)).astype(
        ml_dtypes.bfloat16
    )
    in_maps = []
    for cidx in range(N_CORES):
        X = np.zeros((KP, Q), ml_dtypes.bfloat16)
        base = cidx * BC
        for g in range(G):
            v = min(Q, BC - g * Q)
            X[FMAX * g:FMAX * g + FMAX, :v] = x13[base + g * Q:base + g * Q + v, :].T
        in_maps.append({"xg": X, "w1d": w1, "w2d": w2, "cst": cst})
    return in_maps


def run(x, kernels, biases, final_kernel, final_bias, trace=False, **spmd_kwargs):
    if "nc" not in _BUILD_CACHE:
        _BUILD_CACHE["nc"] = build_program()
    nc = _BUILD_CACHE["nc"]

    fb = float(np.asarray(final_bias).reshape(-1)[0])
    in_maps = _pack_host_inputs(x, kernels, biases, final_kernel)
    for m in in_maps:
        m["cst"][:, 1] = fb
    res = run_bass_kernel_spmd(
        nc, in_maps, list(range(N_CORES)), trace=trace, **spmd_kwargs
    )
    outs = []
    nmac = NBURST // ZMACRO
    for cidx in range(N_CORES):
        op = np.asarray(res.results[cidx]["out"], np.float32)  # (128, nmac*N)
        arr = op.reshape(4, 32, nmac, N).transpose(1, 2, 0, 3)  # (r, m, c, n)
        outs.append(arr[:G].reshape(-1)[:BC])
    y = np.concatenate(outs).reshape(B_TOTAL, 1)
    return y, res


def kernel(x, kernels, biases, final_kernel, final_bias):
    y, _ = run(x, kernels, biases, final_kernel, final_bias, trace=False)
    return y
